# revision 47
# baseline (speedup 1.0000x reference)
"""Trainium2 Bass kernel for an enhanced transformer block (attn + depthwise-conv + MLP).

Sharding: 8 cores = 4 batches x 2 sequence halves (data parallel, no collectives).
Each core receives its batch's x TRANSPOSED (feature-major: d on partitions,
tokens on the free axis) and ROTATED so that its extended token range
[t0-1, t1+1) lands at columns [0, 1026) uniformly on every core. K/V are
computed over the full (rotated) sequence; q/attention only over the core's
1026 extended columns. Halo columns provide the depthwise-conv neighbor
values; at sequence edges the halo is dead (wrapped garbage) and is zeroed
via a mask folded into LN2's rstd.

Datatypes: residual stream and LN intermediates in bf16 (DVE 2x modes);
weights and matmul operands in fp8e4 (x16 scale) using DoubleRow perf mode
(2 contraction tiles per pass at 0.5 cycles/row). Scores matmuls use a
"folded" K/Q layout (32 partitions, head-dim split 2x32 into the DoubleRow
pair axis) built by SBUF->SBUF DMA reshuffle. Softmax has no max-subtraction
(scores are O(1)); denominators ride an extra all-ones column in V through
the P@V DoubleRow matmul. exp() runs on the Act engine (the kernel's floor:
~16.8M elements/core); everything else is kept off Act.
"""

import numpy as np
import ml_dtypes

import concourse.bass as bass
import concourse.bacc as bacc
import concourse.mybir as mybir
import concourse.tile as tile
from concourse.bass_utils import run_bass_kernel_spmd

F32 = mybir.dt.float32
BF16 = mybir.dt.bfloat16
FP8 = mybir.dt.float8e4
Alu = mybir.AluOpType
Act = mybir.ActivationFunctionType
PM = mybir.MatmulPerfMode

D = 512          # model dim
S = 2048         # sequence length
B = 4            # batch
H = 8            # heads
HD = 64          # head dim
DFF = 2048       # mlp hidden
NCORES = 8
TLOC = 1024      # local tokens per core
TEXT = 1026      # extended (1 halo col each side)
QPAD = 1040      # q8 free padded to %16
DT = 4           # d-tiles of 128
EPS = 1e-5
WS = 16.0        # fp8 weight scale
EXPS = 0.125 / (WS * WS)   # exp scale: 1/sqrt(hd) with q,k each carrying WS

# order of packed 512-length vectors in the "vecs" input
VEC_NAMES = ["ln1_g", "ln1_b", "ln2_g", "ln2_b", "lnc_g", "lnc_b",
             "ln3_g", "ln3_b", "cw0", "cw1", "cw2", "cb",
             "bo_eff", "bq", "bk", "b2"]
VIDX = {n: i for i, n in enumerate(VEC_NAMES)}


def _vap(vecs_sb, name, dt):
    """per-partition [128,1] scalar AP for vector `name`, d-tile dt."""
    c = 4 * VIDX[name] + dt
    return vecs_sb[:, c:c + 1]


def build_program(flags, stage=9):
    nc = bacc.Bacc("TRN2", target_bir_lowering=False, debug=False)

    xTb_d = nc.dram_tensor("xTb", (DT, 128, S), BF16, kind="ExternalInput").ap()
    wqkv8_d = nc.dram_tensor("wqkv8", (2, 128, 2, 3 * D), FP8, kind="ExternalInput").ap()
    wo8_d = nc.dram_tensor("wo8", (64, 8, D), FP8, kind="ExternalInput").ap()
    w18_d = nc.dram_tensor("w18", (2, 128, 2, DFF), FP8, kind="ExternalInput").ap()
    w28_d = nc.dram_tensor("w28", (128, 16, D), FP8, kind="ExternalInput").ap()
    vecs_d = nc.dram_tensor("vecs", (128, 4 * len(VEC_NAMES)), F32, kind="ExternalInput").ap()
    b1m_d = nc.dram_tensor("b1m", (128, 16), F32, kind="ExternalInput").ap()
    mask_d = nc.dram_tensor("mask", (128, TEXT), BF16, kind="ExternalInput").ap()
    yT_d = nc.dram_tensor("yT", (DT, 128, TLOC), F32, kind="ExternalOutput").ap()

    with tile.TileContext(nc) as tc:
        _prog(nc, tc, flags, xTb_d, wqkv8_d, wo8_d, w18_d, w28_d, vecs_d,
              b1m_d, mask_d, yT_d, stage=stage)
    nc.compile()
    return nc


def _ln_stats(nc, ps_pool, lnw, ones_bf, z_tiles, sl, n, sq_engines):
    """LN stats over d (128 parts x 4 tiles), token cols `sl` (len n).
    z_tiles: 4 bf16 SBUF tiles/APs. Returns (s1, s2) psum tiles (128, n).
    sq_engines: list of 4 engines for the square ops ('v','a','g')."""
    s1 = ps_pool.tile((128, 512), F32, name="s1", tag="s1", bufs=2)
    s2 = ps_pool.tile((128, 512), F32, name="s2", tag="s2", bufs=2)
    for dt in range(DT):
        sq = lnw.tile((128, 512), BF16, name="sq", tag="sq", bufs=3)
        e = sq_engines[dt]
        if e == "a":
            nc.scalar.square(sq[:, :n], z_tiles[dt][:, sl])
        else:
            eng = nc.vector if e == "v" else nc.gpsimd
            eng.tensor_tensor(sq[:, :n], z_tiles[dt][:, sl],
                              z_tiles[dt][:, sl], Alu.mult)
        nc.tensor.matmul(s1[:, :n], lhsT=ones_bf, rhs=z_tiles[dt][:, sl],
                         start=(dt == 0), stop=(dt == DT - 1))
        nc.tensor.matmul(s2[:, :n], lhsT=ones_bf, rhs=sq[:, :n],
                         start=(dt == 0), stop=(dt == DT - 1))
    return s1, s2


def _ln_murec(nc, lnw, eps_sb, s1, s2, n, mask=None):
    """mu (bf16) and rstd (bf16) from stats psums. mask: optional bf16 AP
    multiplied into rstd (dead-halo kill)."""
    mu = lnw.tile((128, 512), BF16, name="mu", tag="mu", bufs=2)
    nc.scalar.activation(mu[:, :n], s1[:, :n], Act.Copy, scale=1.0 / D)
    mu2 = lnw.tile((128, 512), BF16, name="mu2", tag="mu2", bufs=2)
    nc.vector.tensor_tensor(mu2[:, :n], mu[:, :n], mu[:, :n], Alu.mult)
    var = lnw.tile((128, 512), BF16, name="var", tag="var", bufs=2)
    nc.vector.scalar_tensor_tensor(out=var[:, :n], in0=s2[:, :n],
                                   scalar=1.0 / D, in1=mu2[:, :n],
                                   op0=Alu.mult, op1=Alu.subtract)
    sd = lnw.tile((128, 512), BF16, name="sd", tag="sd", bufs=2)
    nc.scalar.activation(sd[:, :n], var[:, :n], Act.Sqrt, bias=eps_sb[:, 0:1])
    r = lnw.tile((128, 512), BF16, name="r", tag="r", bufs=2)
    with nc.allow_low_precision("bf16 rstd (0.4% on normalized values)"):
        nc.vector.reciprocal(r[:, :n], sd[:, :n])
    if mask is not None:
        nc.vector.tensor_tensor(r[:, :n], r[:, :n], mask, Alu.mult)
    return mu, r


def _ln_apply(nc, lnw, vecs_sb, z_tiles, sl, n, mu, r, out_cb,
              gname, bname, gflag, bflag):
    """out = (z - mu) * r [* g] [+ b]; out written via out_cb(dt, src_ap, n).
    src is a (128, n) bf16 intermediate; out_cb emits the final op."""
    for dt in range(DT):
        xc = lnw.tile((128, 512), BF16, name="xc", tag="xc", bufs=3)
        nc.vector.tensor_tensor(xc[:, :n], z_tiles[dt][:, sl], mu[:, :n],
                                Alu.subtract)
        if gflag:
            rg = lnw.tile((128, 512), BF16, name="rg", tag="rg", bufs=2)
            nc.vector.tensor_scalar_mul(out=rg[:, :n], in0=r[:, :n],
                                        scalar1=_vap(vecs_sb, gname, dt))
            rr = rg
        else:
            rr = r
        out_cb(dt, xc, rr, n)
        if bflag:
            raise NotImplementedError("ln bias folding handled by caller")


def _ln_stats_w(nc, ps_pool, lnw, ones_bf, z_w, sl, n, sbufs=2):
    """Wide LN stats: z_w is a (128, DT, cols) bf16 tile. One wide square op;
    per-dt stats matmuls. Returns (s1, s2) psum (128, n)."""
    s1 = ps_pool.tile((128, 512), F32, name="s1", tag="s1", bufs=sbufs)
    s2 = ps_pool.tile((128, 512), F32, name="s2", tag="s2", bufs=sbufs)
    sqw = lnw.tile((128, DT, 512), BF16, name="sqw", tag="sqw", bufs=2)
    for dt in range(DT):
        if dt % 2 == 0:
            nc.scalar.square(sqw[:, dt, :n], z_w[:, dt, sl])
        else:
            nc.vector.tensor_tensor(sqw[:, dt, :n], z_w[:, dt, sl],
                                    z_w[:, dt, sl], Alu.mult)
    for dt in range(DT):
        nc.tensor.matmul(s1[:, :n], lhsT=ones_bf, rhs=z_w[:, dt, sl],
                         start=(dt == 0), stop=(dt == DT - 1))
        nc.tensor.matmul(s2[:, :n], lhsT=ones_bf, rhs=sqw[:, dt, :n],
                         start=(dt == 0), stop=(dt == DT - 1))
    return s1, s2


def _ln_apply_w(nc, lnw, z_w, sl, n, mu, r, out_ap, out_eng=None):
    """Wide LN apply: out_ap[(128, DT, n)] = (z_w[:, :, sl] - mu_b) * r_b.
    sub on DVE; mul split between DVE (dt 0:2) and Pool (dt 2:4)."""
    mu_b = mu[:, None, :n].broadcast_to((128, DT, n))
    r_b2 = r[:, None, :n].broadcast_to((128, 2, n))
    xcw = lnw.tile((128, DT, 512), BF16, name="xcw", tag="xcw", bufs=1)
    nc.vector.tensor_tensor(xcw[:, :, :n], z_w[:, :, sl], mu_b, Alu.subtract)
    nc.vector.tensor_tensor(out_ap[:, 0:2], xcw[:, 0:2, :n], r_b2, Alu.mult)
    nc.gpsimd.tensor_tensor(out_ap[:, 2:4], xcw[:, 2:4, :n], r_b2, Alu.mult)


def _prog(nc, tc, fl, xTb_d, wqkv8_d, wo8_d, w18_d, w28_d, vecs_d, b1m_d,
          mask_d, yT_d, stage=9):
    for nm in ("ln1_b", "ln2_b", "lnc_b", "ln3_b"):
        assert not fl[nm], "LN biases unimplemented in fast path"
    for nm in ("ln2_g", "lnc_g", "ln3_g"):
        assert not fl[nm], "LN gains unimplemented in wide tail path"

    Ls, Rs, Ps = [], [], []

    def _rel(lst, pool):
        lst.remove(pool)
        pool.release()

    def _dbg_exit(tiles, conv=True):
        dbg = tc.alloc_tile_pool(name="dbgout", bufs=1)
        for dt in range(DT):
            t = dbg.tile((128, TLOC), F32, name=f"dbg{dt}", tag=f"dbg{dt}")
            nc.vector.tensor_copy(t, tiles[dt][:, 0:TLOC])
            nc.sync.dma_start(out=yT_d[dt], in_=t)
        dbg.release()
        for st in (Ps, Ls, Rs):
            while st:
                st.pop().release()


    # ---------------- persistent pools (alloc order = reverse release) ----
    consts = tc.alloc_tile_pool(name="consts", bufs=1); Ls.append(consts)
    wts = tc.alloc_tile_pool(name="wts", bufs=1); Ls.append(wts)
    lnw = tc.alloc_tile_pool(name="lnw", bufs=2); Ls.append(lnw)
    mlpp = tc.alloc_tile_pool(name="mlpp", bufs=1); Ls.append(mlpp)
    conv_t = tc.alloc_tile_pool(name="conv_t", bufs=1); Ls.append(conv_t)
    mid = tc.alloc_tile_pool(name="mid", bufs=1); Ls.append(mid)

    # x tile DMA issued first (LN1's critical path)
    x_pool = tc.alloc_tile_pool(name="x_pool", bufs=1); Ls.append(x_pool)
    x_all = x_pool.tile((128, DT, S), BF16, name="x_all", tag="x_all")
    for ch in range(4):
        sl = slice(ch * 512, ch * 512 + 512)
        nc.sync.dma_start(out=x_all[:, :, sl],
                          in_=xTb_d.rearrange("d p t -> p d t")[:, :, sl])
    x_sb = [x_all[:, dt] for dt in range(DT)]

    vecs_sb = consts.tile((128, 4 * len(VEC_NAMES)), F32, name="vecs_sb", tag="vecs")
    nc.sync.dma_start(out=vecs_sb, in_=vecs_d)
    b1_sb = consts.tile((128, 16), F32, name="b1_sb", tag="b1")
    nc.sync.dma_start(out=b1_sb, in_=b1m_d)
    mask_sb = consts.tile((128, TEXT), BF16, name="mask_sb", tag="mask")
    nc.sync.dma_start(out=mask_sb, in_=mask_d)
    ones_bf = consts.tile((128, 128), BF16, name="ones_bf", tag="ones")
    nc.vector.memset(ones_bf, 1.0)
    ows = consts.tile((1, 64), BF16, name="ows", tag="ows")   # 1/WS row
    nc.vector.memset(ows, 1.0 / WS)
    eps_sb = consts.tile((128, 1), F32, name="eps_sb", tag="eps")
    nc.vector.memset(eps_sb, EPS)
    c16 = consts.tile((128, 1), F32, name="c16", tag="c16")   # 1/WS scalar
    nc.vector.memset(c16, 1.0 / WS)

    wqkv_sb = []
    for p in range(2):
        t = wts.tile((128, 2, 3 * D), FP8, name=f"wqkv{p}", tag=f"wqkv{p}")
        nc.sync.dma_start(out=t, in_=wqkv8_d[p])
        wqkv_sb.append(t)
    wo_sb = wts.tile((64, 8, D), FP8, name="wo8", tag="wo8")
    nc.sync.dma_start(out=wo_sb, in_=wo8_d)

    # a8: head-major fp8 attention output (64 d-rows, per head 1040 cols)
    a_pool = tc.alloc_tile_pool(name="a_pool", bufs=1); Ls.append(a_pool)
    a8 = a_pool.tile((64, 8, QPAD), FP8, name="a8", tag="a8")

    # folded k8/q8 + v8 - attention inputs
    kvq = tc.alloc_tile_pool(name="kvq", bufs=1); Ls.append(kvq)
    k8 = [kvq.tile((96, 2, S), FP8, name=f"k8_{g}", tag=f"k8_{g}")
          for g in range(3)]
    q8 = [kvq.tile((96, 2, QPAD), FP8, name=f"q8_{g}", tag=f"q8_{g}")
          for g in range(3)]
    v8 = kvq.tile((128, 16, 8, 66), FP8, name="v8", tag="v8")
    nc.vector.memset(v8[:, :, :, 64:65], 1.0)

    # pt tiles (attention P pairs) - allocated below h_pool so attention
    # doesn't wait on h_pool release
    p_pool = tc.alloc_tile_pool(name="p_pool", bufs=2); Ls.append(p_pool)

    # h8 (LN1 out, fp8) + k/q staging (feature-major fp8) - until QKV done
    h_pool = tc.alloc_tile_pool(name="h_pool", bufs=1); Ls.append(h_pool)
    h8 = h_pool.tile((128, DT, S), FP8, name="h8", tag="h8")
    k_st = [h_pool.tile((128, S), FP8, name=f"kst{jt}", tag=f"kst{jt}")
            for jt in range(DT)]
    q_st = [h_pool.tile((128, TEXT), FP8, name=f"qst{jt}", tag=f"qst{jt}")
            for jt in range(DT)]

    # ---------------- phase 1+2+3: LN1 + QKV + attention (interleaved) -----
    ln1ps = tc.alloc_tile_pool(name="ln1ps", bufs=1, space="PSUM"); Ps.append(ln1ps)  # s1/s2 2 banks
    qkvps = tc.alloc_tile_pool(name="qkvps", bufs=2, space="PSUM"); Ps.append(qkvps)

    def h_pair(p, sl):
        return h8[:, 2 * p:2 * p + 2, sl]

    assert not fl["ln1_g"], "ln1 gain unimplemented in wide path"
    cp_rr = [0]

    def cp_psum(dst, ps_ap):
        """round-robin psum->sbuf copy on DVE/Act"""
        if cp_rr[0] % 2 == 0:
            nc.vector.tensor_copy(dst, ps_ap)
        else:
            nc.scalar.copy(dst, ps_ap)
        cp_rr[0] += 1

    with nc.named_scope("ln1qkv"):
        murs = {}

        def ln1_stats_murec(ch):
            sl = slice(ch * 512, ch * 512 + 512)
            s1, s2 = _ln_stats_w(nc, ln1ps, lnw, ones_bf, x_all, sl, 512, sbufs=1)
            murs[ch] = _ln_murec(nc, lnw, eps_sb, s1, s2, 512)

        def ln1_apply_qkv(ch):
            sl = slice(ch * 512, ch * 512 + 512)
            mu, r = murs.pop(ch)
            mu_b = mu[:, None, :].broadcast_to((128, DT, 512))
            r_b2 = r[:, None, :].broadcast_to((128, 2, 512))
            xcw = lnw.tile((128, DT, 512), BF16, name="xcw", tag="xcw", bufs=1)
            nc.vector.tensor_tensor(xcw, x_all[:, :, sl], mu_b, Alu.subtract)
            nc.vector.tensor_tensor(h8[:, 0:2, sl], xcw[:, 0:2, :], r_b2,
                                    Alu.mult)
            nc.gpsimd.tensor_tensor(h8[:, 2:4, sl], xcw[:, 2:4, :], r_b2,
                                    Alu.mult)
            # v for this chunk's 4 token-tiles (paired psum, 2 copies)
            for tp in range(2):
                ps = qkvps.tile((128, 1024), F32, name="vps", tag="vmm",
                                bufs=2)
                for half in range(2):
                    tc_ = 4 * ch + 2 * tp + half
                    hs = slice(half * 512, half * 512 + 512)
                    for p in range(2):
                        nc.tensor.matmul(ps[:, hs],
                                         lhsT=h_pair(p, slice(tc_ * 128, tc_ * 128 + 128)),
                                         rhs=wqkv_sb[p][:, :, 2 * D:3 * D],
                                         start=(p == 0), stop=(p == 1),
                                         perf_mode=PM.DoubleRow)
                nc.scalar.copy(
                    v8[:, 4 * ch + 2 * tp:4 * ch + 2 * tp + 2, :, 0:64],
                    ps[:, :].rearrange("p (tp2 h d) -> p tp2 h d", tp2=2, h=H))
            # k for this chunk, all jt
            for jt in range(DT):
                ps = qkvps.tile((128, 512), F32, name="kps", tag="mm")
                for p in range(2):
                    nc.tensor.matmul(ps, lhsT=wqkv_sb[p][:, :, D + jt * 128:D + jt * 128 + 128],
                                     rhs=h_pair(p, sl),
                                     start=(p == 0), stop=(p == 1),
                                     perf_mode=PM.DoubleRow)
                if fl["bk"]:
                    nc.scalar.add(out=k_st[jt][:, sl], in_=ps,
                                  add=_vap(vecs_sb, "bk", jt))
                else:
                    cp_psum(k_st[jt][:, sl], ps)  # round-robin DVE/Act
            # q: chunk 0/1 are cols 0:512/512:1024; tiny halo cols in ch2
            if ch < 2:
                qc0, qn = ch * 512, 512
            elif ch == 2:
                qc0, qn = 1024, 2
            else:
                qc0 = None
            if qc0 is not None:
                for jt in range(DT):
                    ps = qkvps.tile((128, 512), F32, name="qps", tag="mm")
                    for p in range(2):
                        nc.tensor.matmul(ps[:, :qn],
                                         lhsT=wqkv_sb[p][:, :, jt * 128:jt * 128 + 128],
                                         rhs=h_pair(p, slice(qc0, qc0 + qn)),
                                         start=(p == 0), stop=(p == 1),
                                         perf_mode=PM.DoubleRow)
                    if fl["bq"]:
                        nc.scalar.add(out=q_st[jt][:, qc0:qc0 + qn],
                                      in_=ps[:, :qn],
                                      add=_vap(vecs_sb, "bq", jt))
                    else:
                        nc.vector.tensor_copy(q_st[jt][:, qc0:qc0 + qn],
                                              ps[:, :qn])

        # software pipeline: chunk c's apply/qkv emitted after chunk c+1's
        # stats+murec so murec chains never queue behind heavy applies
        ln1_stats_murec(0)
        for ch in range(1, 4):
            ln1_stats_murec(ch)
            ln1_apply_qkv(ch - 1)
        ln1_apply_qkv(3)
        # fold into k8/q8 (head h = 2*jt + i at k_st[jt] rows i*64 + s*32)
        for jt in range(DT):
            for i in range(2):
                h = 2 * jt + i
                g, po = h // 3, (h % 3) * 32
                for s_ in range(2):
                    r0 = i * 64 + s_ * 32
                    nc.sync.dma_start(out=k8[g][po:po + 32, s_, :],
                                      in_=k_st[jt][r0:r0 + 32, :])
                    nc.sync.dma_start(out=q8[g][po:po + 32, s_, 0:TEXT],
                                      in_=q_st[jt][r0:r0 + 32, :])
    w1_sb = []
    for p in range(2):
        t = wts.tile((128, 2, DFF), FP8, name=f"w1_{p}", tag=f"w1_{p}")
        nc.sync.dma_start(out=t, in_=w18_d[p])
        w1_sb.append(t)
    w2_sb = wts.tile((128, 16, D), FP8, name="w2_8", tag="w2_8")
    nc.sync.dma_start(out=w2_sb, in_=w28_d)
    _rel(Ps, qkvps)
    _rel(Ps, ln1ps)
    _rel(Ls, h_pool)

    if stage == 1:
        dbg = [lnw.tile((128, TLOC), F32, name=f"s1d{dt}", tag=f"s1d{dt}")
               for dt in range(DT)]
        for dt in range(DT):
            nc.vector.tensor_copy(dbg[dt], h8[:, dt, 0:TLOC])
        return _dbg_exit(dbg)
    if stage == 2:
        dbg = [lnw.tile((128, TLOC), F32, name=f"s2d{dt}", tag=f"s2d{dt}")
               for dt in range(DT)]
        for dt in range(DT):
            nc.vector.tensor_copy(dbg[dt], k_st[0][:, 0:TLOC])
        return _dbg_exit(dbg)

    scps = tc.alloc_tile_pool(name="scps", bufs=2, space="PSUM"); Ps.append(scps)
    avps = tc.alloc_tile_pool(name="avps", bufs=2, space="PSUM"); Ps.append(avps)

    def emit_attn(h):
        with nc.named_scope(f"attn{h}"):
            g, po = h // 3, (h % 3) * 32
            av = avps.tile((128, 1024), F32, name=f"av{h}", tag="av")
            pt = p_pool.tile((128, 4, 1024), FP8, name=f"pt{h}", tag="pt")
            for kc in range(16):
                sc = scps.tile((128, 1024), F32, name="sc", tag="sc")
                ksl = slice(kc * 128, kc * 128 + 128)
                for qc in range(2):
                    qsl = slice(qc * 512, qc * 512 + 512)
                    nc.tensor.matmul(sc[:, qsl],
                                     lhsT=k8[g][po:po + 32, :, ksl],
                                     rhs=q8[g][po:po + 32, :, qsl],
                                     start=True, stop=True,
                                     perf_mode=PM.DoubleRow)
                nc.scalar.activation(pt[:, kc % 4, :], sc, Act.Exp, scale=EXPS)
                if kc % 2 == 1:
                    pr = kc // 2
                    s0 = (2 * pr) % 4
                    for qc in range(2):
                        qsl = slice(qc * 512, qc * 512 + 512)
                        nc.tensor.matmul(av[0:65, qsl],
                                         lhsT=v8[:, 2 * pr:2 * pr + 2, h, 0:65],
                                         rhs=pt[:, s0:s0 + 2, qsl],
                                         start=(pr == 0), stop=(pr == 7),
                                         perf_mode=PM.DoubleRow)
            # normalize (pipelined qc halves): rec=1/denom; rrep=(rec/WS)
            for qc in range(2):
                qsl = slice(qc * 512, qc * 512 + 512)
                rec = lnw.tile((1, 512), BF16, name="rec", tag="rec", bufs=2)
                with nc.allow_low_precision("bf16 softmax denom recip"):
                    nc.vector.reciprocal(rec, av[64:65, qsl])
                nc.tensor.matmul(av[64:128, qsl], lhsT=ows, rhs=rec,
                                 start=True, stop=True)
                rrep = lnw.tile((64, 512), BF16, name="rrep", tag="rrep",
                                bufs=2)
                nc.vector.tensor_copy(rrep, av[64:128, qsl])
                nc.vector.tensor_tensor(a8[:, h, qsl], av[0:64, qsl], rrep,
                                        Alu.mult)

    for h in range(H):
        emit_attn(h)

    _rel(Ps, avps); _rel(Ps, scps)
    _rel(Ls, p_pool)

    # ---------------- phase 3b: halo attention (2 ext cols, batched) -------
    hps = tc.alloc_tile_pool(name="hps", bufs=1, space="PSUM"); Ps.append(hps)
    hsb = tc.alloc_tile_pool(name="hsb", bufs=1)
    with nc.named_scope("halo"):
        hsc = hps.tile((128, 8, 16, 2), F32, name="hsc", tag="hsc")
        for h in range(H):
            g, po = h // 3, (h % 3) * 32
            for kc in range(16):
                nc.tensor.matmul(hsc[:, h, kc, :],
                                 lhsT=k8[g][po:po + 32, :, kc * 128:kc * 128 + 128],
                                 rhs=q8[g][po:po + 32, :, 1024:1026],
                                 start=True, stop=True, perf_mode=PM.DoubleRow)
        ph8 = hsb.tile((128, 8, 16, 2), FP8, name="ph8", tag="ph8")
        nc.scalar.activation(ph8[:, :, :, :], hsc[:, :, :, :], Act.Exp, scale=EXPS)
        avh = hps.tile((128, 16), F32, name="avh", tag="avh")
        for h in range(H):
            for pr in range(8):
                nc.tensor.matmul(avh[0:65, 2 * h:2 * h + 2],
                                 lhsT=v8[:, 2 * pr:2 * pr + 2, h, 0:65],
                                 rhs=ph8[:, h, 2 * pr:2 * pr + 2, :],
                                 start=(pr == 0), stop=(pr == 7),
                                 perf_mode=PM.DoubleRow)
        rec2 = hsb.tile((1, 16), BF16, name="rec2", tag="rec2")
        with nc.allow_low_precision("bf16 halo denom recip"):
            nc.vector.reciprocal(rec2, avh[64:65, :])
        nc.tensor.matmul(avh[64:128, :], lhsT=ows, rhs=rec2,
                         start=True, stop=True)
        rr2 = hsb.tile((64, 16), BF16, name="rr2", tag="rr2")
        nc.vector.tensor_copy(rr2, avh[64:128, :])
        ah8 = hsb.tile((64, 16), FP8, name="ah8", tag="ah8")
        nc.vector.tensor_tensor(ah8, avh[0:64, :], rr2, Alu.mult)
        for h in range(H):
            nc.vector.tensor_copy(a8[:, h, 1024:1026], ah8[:, 2 * h:2 * h + 2])
    hsb.release()
    _rel(Ps, hps)
    _rel(Ls, kvq)

    if stage == 3:
        dbg = [lnw.tile((128, TLOC), F32, name=f"s3d{dt}", tag=f"s3d{dt}")
               for dt in range(DT)]
        for dt in range(DT):
            for i in range(2):
                nc.vector.tensor_copy(dbg[dt][i * 64:i * 64 + 64, :],
                                      a8[:, 2 * dt + i, 0:TLOC])
        return _dbg_exit(dbg)

    # ---------------- phase 4+5: out-proj -> x1 -> conv block (wavefront) --
    x1w = mid.tile((128, DT, TEXT), BF16, name="x1w", tag="x1w")
    x1_sb = [x1w[:, dt] for dt in range(DT)]
    h2w = mid.tile((128, DT, TEXT), BF16, name="h2w", tag="h2w")
    tcw = conv_t.tile((128, DT, TLOC), BF16, name="tcw", tag="tcw")
    tcv = [tcw[:, dt] for dt in range(DT)]
    x2w = conv_t.tile((128, DT, TLOC), BF16, name="x2w", tag="x2w")
    x2_sb = [x2w[:, dt] for dt in range(DT)]

    cps = tc.alloc_tile_pool(name="cps", bufs=1, space="PSUM"); Ps.append(cps)
    ops = tc.alloc_tile_pool(name="ops", bufs=4, space="PSUM"); Ps.append(ops)
    QC3 = ((0, 512), (512, 512), (1024, 2))   # sliver last (waits halo)
    with nc.named_scope("outproj_ln2"):
        oi = 0
        for (c0, n) in QC3:
            sl = slice(c0, c0 + n)
            for jt in range(DT):
                ps = ops.tile((128, 512), F32, name="ops_t", tag="o")
                for p in range(4):
                    nc.tensor.matmul(ps[:, :n],
                                     lhsT=wo_sb[:, 2 * p:2 * p + 2, jt * 128:jt * 128 + 128],
                                     rhs=a8[:, 2 * p:2 * p + 2, sl],
                                     start=(p == 0), stop=(p == 3),
                                     perf_mode=PM.DoubleRow)
                if oi % 2 == 0 or n != 512:
                    nc.vector.scalar_tensor_tensor(out=x1w[:, jt, sl],
                                                   in0=ps[:, :n],
                                                   scalar=c16[:, 0:1],
                                                   in1=x_sb[jt][:, sl],
                                                   op0=Alu.mult, op1=Alu.add)
                else:
                    t = lnw.tile((128, 512), BF16, name="opt", tag="opt",
                                 bufs=2)
                    nc.scalar.activation(t[:, :n], ps[:, :n], Act.Copy,
                                         scale=1.0 / WS)
                    nc.vector.tensor_tensor(x1w[:, jt, sl], t[:, :n],
                                            x_sb[jt][:, sl], Alu.add)
                if fl["bo"]:
                    nc.vector.tensor_scalar_add(out=x1w[:, jt, sl],
                                                in0=x1w[:, jt, sl],
                                                scalar1=_vap(vecs_sb, "bo_eff", jt))
                oi += 1
            # LN2 for this chunk immediately (wavefront)
            s1, s2 = _ln_stats_w(nc, cps, lnw, ones_bf, x1w, sl, n)
            mu, r = _ln_murec(nc, lnw, eps_sb, s1, s2, n, mask=mask_sb[:, sl])
            _ln_apply_w(nc, lnw, x1w, sl, n, mu, r, h2w[:, :, sl])
    _rel(Ps, ops)
    _rel(Ls, a_pool)
    _rel(Ls, x_pool)
    if stage == 4:
        return _dbg_exit(x1_sb)

    h38 = mlpp.tile((128, DT, TLOC), FP8, name="h38", tag="h38")
    with nc.named_scope("convblock"):
        gw = conv_t.tile((128, DT, TLOC), BF16, name="gw", tag="gw")
        t0w = x2w  # scratch reuse: x2 adds overwrite after conv consumed it
        t1w = gw   # scratch reuse: gelu output overwrites after conv adds
        for ch in range(2):
            c0 = ch * 512
            sl = slice(c0, c0 + 512)
            for dt in range(DT):
                nc.vector.tensor_scalar(out=t0w[:, dt, sl],
                                        in0=h2w[:, dt, c0:c0 + 512],
                                        scalar1=_vap(vecs_sb, "cw0", dt),
                                        scalar2=_vap(vecs_sb, "cb", dt),
                                        op0=Alu.mult, op1=Alu.add)
                nc.scalar.activation(t1w[:, dt, sl],
                                     h2w[:, dt, c0 + 1:c0 + 513], Act.Copy,
                                     scale=_vap(vecs_sb, "cw1", dt))
                nc.vector.tensor_scalar_mul(out=tcw[:, dt, sl],
                                            in0=h2w[:, dt, c0 + 2:c0 + 514],
                                            scalar1=_vap(vecs_sb, "cw2", dt))
            nc.vector.tensor_tensor(t0w[:, :, sl], t0w[:, :, sl],
                                    t1w[:, :, sl], Alu.add)
            nc.vector.tensor_tensor(tcw[:, :, sl], tcw[:, :, sl],
                                    t0w[:, :, sl], Alu.add)
            s1, s2 = _ln_stats_w(nc, cps, lnw, ones_bf, tcw, sl, 512)
            mu, r = _ln_murec(nc, lnw, eps_sb, s1, s2, 512)
            _ln_apply_w(nc, lnw, tcw, sl, 512, mu, r, tcw[:, :, sl])
            for half in range(2):
                dsl = slice(half * 2, half * 2 + 2)
                nc.scalar.activation(gw[:, dsl, sl], tcw[:, dsl, sl], Act.Gelu)
            nc.vector.tensor_tensor(x2w[:, :, sl],
                                    x1w[:, :, c0 + 1:c0 + 513],
                                    h2w[:, :, c0 + 1:c0 + 513],
                                    Alu.add)
            nc.vector.tensor_tensor(x2w[:, :, sl], x2w[:, :, sl],
                                    gw[:, :, sl], Alu.add)
        # LN3 after both conv chunks (separate stage avoids DVE HOL on conv c1)
        for ch in range(2):
            sl = slice(ch * 512, ch * 512 + 512)
            s1, s2 = _ln_stats_w(nc, cps, lnw, ones_bf, x2w, sl, 512)
            mu, r = _ln_murec(nc, lnw, eps_sb, s1, s2, 512)
            _ln_apply_w(nc, lnw, x2w, sl, 512, mu, r, h38[:, :, sl])
    _rel(Ps, cps)
    if stage == 5:
        return _dbg_exit(x2_sb)

    # ---------------- phase 6: MLP -> output ----------------
    u8 = mlpp.tile((128, 16, TLOC), FP8, name="u8", tag="u8")

    mps = tc.alloc_tile_pool(name="mps", bufs=2, space="PSUM"); Ps.append(mps)
    with nc.named_scope("mlp"):
        def h3_pair(p, sl):
            return h38[:, 2 * p:2 * p + 2, sl]

        for jt in range(16):
            ups = mps.tile((128, 1024), F32, name="ups", tag="ups")
            for ch in range(2):
                sl = slice(ch * 512, ch * 512 + 512)
                for p in range(2):
                    nc.tensor.matmul(ups[:, sl],
                                     lhsT=w1_sb[p][:, :, jt * 128:jt * 128 + 128],
                                     rhs=h3_pair(p, sl),
                                     start=(p == 0), stop=(p == 1),
                                     perf_mode=PM.DoubleRow)
            if fl["b1"]:
                nc.scalar.activation(u8[:, jt, :], ups, Act.Gelu,
                                     bias=b1_sb[:, jt:jt + 1], scale=1.0 / WS)
            else:
                nc.scalar.activation(u8[:, jt, :], ups, Act.Gelu, scale=1.0 / WS)

    _rel(Ps, mps)
    yps_pool = tc.alloc_tile_pool(name="yps_pool", bufs=3, space="PSUM"); Ps.append(yps_pool)
    with nc.named_scope("mlp2"):
        for jt in range(DT):
            out_t = mlpp.tile((128, TLOC), F32, name="out_t", tag="out_t",
                              bufs=2)
            for ch in range(2):
                sl = slice(ch * 512, ch * 512 + 512)
                ps = yps_pool.tile((128, 512), F32, name="yps", tag="yps")
                for p in range(8):
                    nc.tensor.matmul(ps,
                                     lhsT=w2_sb[:, 2 * p:2 * p + 2, jt * 128:jt * 128 + 128],
                                     rhs=u8[:, 2 * p:2 * p + 2, sl],
                                     start=(p == 0), stop=(p == 7),
                                     perf_mode=PM.DoubleRow)
                eng = nc.vector if ch == 0 else nc.gpsimd
                if eng is nc.gpsimd:
                    eng = nc.vector  # gpsimd cannot read PSUM
                eng.scalar_tensor_tensor(out=out_t[:, sl], in0=ps,
                                         scalar=c16[:, 0:1],
                                         in1=x2_sb[jt][:, sl],
                                         op0=Alu.mult, op1=Alu.add)
                if fl["b2"]:
                    nc.vector.tensor_scalar_add(out=out_t[:, sl],
                                                in0=out_t[:, sl],
                                                scalar1=_vap(vecs_sb, "b2", jt))
            nc.sync.dma_start(out=yT_d[jt], in_=out_t)
    _rel(Ps, yps_pool)
    _rel(Ls, mid)
    _rel(Ls, conv_t)
    _rel(Ls, mlpp)
    _rel(Ls, lnw); _rel(Ls, wts); _rel(Ls, consts)


# ======================= host side =======================

def _nz(a):
    return bool(np.any(np.asarray(a) != 0))


def _q8(w):
    """fp8e4 quantize with WS scale and saturation clip."""
    return np.clip(np.asarray(w, np.float64) * WS, -240, 240).astype(
        ml_dtypes.float8_e4m3)


def prepare(inputs):
    f32 = np.float32
    bf = ml_dtypes.bfloat16
    g = {k: np.asarray(v, f32) for k, v in inputs.items()}
    x = g["x"]
    Wqkv, Wo, W1, W2 = g["Wqkv"], g["Wo"], g["W1"], g["W2"]
    conv_w = g["conv_w"]

    flags = {
        "ln1_g": not np.allclose(g["ln1_g"], 1.0), "ln1_b": _nz(g["ln1_b"]),
        "ln2_g": not np.allclose(g["ln2_g"], 1.0), "ln2_b": _nz(g["ln2_b"]),
        "lnc_g": not np.allclose(g["lnc_g"], 1.0), "lnc_b": _nz(g["lnc_b"]),
        "ln3_g": not np.allclose(g["ln3_g"], 1.0), "ln3_b": _nz(g["ln3_b"]),
        "bq": _nz(g["bqkv"][:D]), "bk": _nz(g["bqkv"][D:2 * D]),
        "cb": _nz(g["conv_b"]),
        "b1": _nz(g["b1"]), "b2": _nz(g["b2"]),
    }
    bv = g["bqkv"][2 * D:]
    bo_eff = g["bo"] + Wo @ bv
    flags["bo"] = _nz(bo_eff)

    # weights, fp8 x16, pair layouts
    WqkvT = np.ascontiguousarray(Wqkv.T)               # (512, 1536)
    wqkv8 = np.zeros((2, 128, 2, 3 * D), ml_dtypes.float8_e4m3)
    for p in range(2):
        for s in range(2):
            blk = (2 * p + s) * 128
            wqkv8[p, :, s, :] = _q8(WqkvT[blk:blk + 128, :])
    WoT = np.ascontiguousarray(Wo.T)                   # (512, 512) [d, j]
    wo8 = np.zeros((64, 8, D), ml_dtypes.float8_e4m3)
    for h in range(8):
        wo8[:, h, :] = _q8(WoT[h * 64:(h + 1) * 64, :])
    W1T = np.ascontiguousarray(W1.T)                   # (512, 2048)
    w18 = np.zeros((2, 128, 2, DFF), ml_dtypes.float8_e4m3)
    for p in range(2):
        for s in range(2):
            blk = (2 * p + s) * 128
            w18[p, :, s, :] = _q8(W1T[blk:blk + 128, :])
    W2T = np.ascontiguousarray(W2.T)                   # (2048, 512)
    w28 = np.zeros((128, 16, D), ml_dtypes.float8_e4m3)
    for t in range(16):
        w28[:, t, :] = _q8(W2T[t * 128:(t + 1) * 128, :])

    shared = {
        "wqkv8": wqkv8, "wo8": wo8, "w18": w18, "w28": w28,
        "b1m": np.ascontiguousarray(g["b1"].reshape(16, 128).T).astype(f32),
    }
    vec_vals = {
        "ln1_g": g["ln1_g"], "ln1_b": g["ln1_b"], "ln2_g": g["ln2_g"],
        "ln2_b": g["ln2_b"], "lnc_g": g["lnc_g"], "lnc_b": g["lnc_b"],
        "ln3_g": g["ln3_g"], "ln3_b": g["ln3_b"],
        "cw0": conv_w[:, 0], "cw1": conv_w[:, 1], "cw2": conv_w[:, 2],
        "cb": g["conv_b"], "bo_eff": bo_eff, "bq": g["bqkv"][:D],
        "bk": g["bqkv"][D:2 * D], "b2": g["b2"],
    }
    vecs = np.zeros((128, 4 * len(VEC_NAMES)), f32)
    for i, nme in enumerate(VEC_NAMES):
        vecs[:, 4 * i:4 * i + 4] = vec_vals[nme].reshape(DT, 128).T
    shared["vecs"] = vecs

    per_core = []
    for c in range(NCORES):
        b, half = c // 2, c % 2
        t0 = half * TLOC
        xT = np.ascontiguousarray(x[b].T)
        xrot = np.roll(xT, -(t0 - 1), axis=1)
        mask = np.ones((128, TEXT), bf)
        if half == 0:
            mask[:, 0] = 0.0
        else:
            mask[:, TEXT - 1] = 0.0
        im = dict(shared)
        im["xTb"] = np.ascontiguousarray(xrot.reshape(DT, 128, S)).astype(bf)
        im["mask"] = mask
        per_core.append(im)
    return flags, per_core


_PROG_CACHE = {}


def get_program(flags, stage=9):
    key = (tuple(sorted(flags.items())), stage)
    if key not in _PROG_CACHE:
        _PROG_CACHE[key] = build_program(flags, stage)
    return _PROG_CACHE[key]


def run(inputs, stage=9, **spmd_kwargs):
    flags, per_core = prepare(inputs)
    nc = get_program(flags, stage)
    res = run_bass_kernel_spmd(nc, per_core, core_ids=list(range(NCORES)),
                               **spmd_kwargs)
    out = np.empty((B, S, D), np.float32)
    for c in range(NCORES):
        b, half = c // 2, c % 2
        t0 = half * TLOC
        yT = res.results[c]["yT"].reshape(D, TLOC)
        out[b, t0:t0 + TLOC, :] = yT.T
    return out, res


def kernel(**inputs) -> np.ndarray:
    out, _ = run(inputs)
    return out
